# revision 8
# baseline (speedup 1.0000x reference)
"""Multi-head attention forward on 8 Trainium2 NeuronCores (Bass/Tile).

Problem: B=2, S=2048, HIDDEN=2048, HEADS=16, D_K=128, fp32 I/O,
mask all-ones (eval). torch-Linear convention: y = x @ W.T.

Sharding (head + output-row parallel, two AllToAlls, no all-reduce):
  core c (0..7) owns heads {2c, 2c+1} for BOTH batches; after attention an
  8-rank AllToAll re-shards from head-split to row-split for the W_o GEMM.

v2 — single continuous PE stream (HAM-warm, no phase barriers):
  - A (projections): ht-inner accumulation chunks (16 matmuls into ONE psum
    bank), streaming x column-blocks. Q,K projected into [d, s] layout
    (W-stationary, N=512); V projected directly into the natural [s, d]
    layout (x-stationary, N=256) - no PE transposes at all.
  - B (attention): per (qb,l): 4 groups of 4 kt; scores matmuls fill a
    4-bank [128, 2048] PSUM tile; ONE big exp on ACT per group; PV
    accumulates in [d, q]. Softmax denominators: DVE pair-tree + Pool
    cross-group adds + one GPSIMD partition_all_reduce (PE never touches
    softmax bookkeeping).
  - Cross-phase interleave in emission (= Tile priority) order:
      A0 dense | B0 x A1 | a2a0 | B1 x D0(half) | a2a1 | D0(rest), D1
    so the PE always has independent matmuls to run while ACT does exp and
    while collectives are in flight.
  - Output written bf16 (halves the tail DMA); host upcasts to f32.
Host side: pre-transpose/cast inputs to bf16, slice weights per core,
scatter-gather the per-core [512, 2048] chunks into the full output.
"""

import math
from contextlib import ExitStack

import ml_dtypes
import numpy as np

import concourse.bass as bass
import concourse.bass_isa as bass_isa
import concourse.tile as tile
from concourse import bacc, mybir
from concourse.bass_utils import run_bass_kernel_spmd

BF16 = mybir.dt.bfloat16
F32 = mybir.dt.float32
NPBF16 = ml_dtypes.bfloat16

HIDDEN = 2048
HEADS = 16
D_K = 128
B = 2
N_CORES = 8
HPC = HEADS // N_CORES          # heads per core (2)
DPC = HPC * D_K                 # concat cols per core (256)
NHT = HIDDEN // 128             # 16 hidden-dim 128-tiles


class ChunkStream:
    """Ordered chunks of (dma_fn|None, compute_fn); emits with prefetch."""

    def __init__(self, chunks, prefetch=2):
        self.chunks = list(chunks)
        self.ci = 0     # next compute
        self.di = 0     # next dma
        self.prefetch = prefetch

    def _run_dmas(self):
        # issue dma for chunks up to `prefetch` compute-chunks ahead
        while self.di < len(self.chunks) and self.di < self.ci + self.prefetch:
            dma_fn, _ = self.chunks[self.di]
            if dma_fn is not None:
                dma_fn()
            self.di += 1

    def emit_one(self, dep=None):
        if self.ci >= len(self.chunks):
            return False
        self._run_dmas()
        self.chunks[self.ci][1](dep=dep)
        self.ci += 1
        self._run_dmas()
        return True

    def drain(self):
        while self.emit_one():
            pass


def _mha_kernel(ctx: ExitStack, tc: tile.TileContext, aps: dict, S: int):
    nc = tc.nc
    NKT = S // 128                   # seq 128-tiles (16)
    QBLK = 512
    NQB = S // QBLK                  # 4
    NSB = S // QBLK                  # x col-blocks per tensor (4)
    GK = 2                           # kt per exp-group
    NG = NKT // GK                   # groups per (qb,l) (8)
    SCB = S // N_CORES               # per-batch output rows per core (256)
    scale = 1.0 / math.sqrt(D_K)

    qT, kT, vT = aps["qT"], aps["kT"], aps["vT"]   # per batch [HIDDEN, S]
    wqT, wkT, wvT = aps["wqT"], aps["wkT"], aps["wvT"]  # [128, NHT*DPC]
    woT = aps["woT"]                                # [128, NHT*HIDDEN]
    out = aps["out"]                                # [B*SCB, HIDDEN] bf16
    a2a_in = aps["a2a_in"]                          # per batch [8*DPC, SCB]
    a2a_out = aps["a2a_out"]                        # per batch [8*DPC, SCB]

    # ---------------- resident weights ----------------
    w_pool = ctx.enter_context(tc.tile_pool(name="wqkv", bufs=1))
    wq_sb = w_pool.tile([128, NHT * DPC], BF16, tag="wq")
    wk_sb = w_pool.tile([128, NHT * DPC], BF16, tag="wk")
    wv_sb = w_pool.tile([128, NHT * DPC], BF16, tag="wv")
    wck = NHT * DPC // 4
    # K is projected first: its weight chunks go out first.  wv/wq are
    # DMAed lazily from the first V/Q chunk so the first K matmul's
    # inputs hit the DMA queues with no competition.
    for ck in range(4):
        nc.sync.dma_start(out=wk_sb[:, ck * wck:(ck + 1) * wck],
                          in_=wkT[:, ck * wck:(ck + 1) * wck])
    def emit_wv_dma():
        for ck in range(4):
            nc.sync.dma_start(out=wv_sb[:, ck * wck:(ck + 1) * wck],
                              in_=wvT[:, ck * wck:(ck + 1) * wck])

    def emit_wq_dma():
        for ck in range(4):
            nc.sync.dma_start(out=wq_sb[:, ck * wck:(ck + 1) * wck],
                              in_=wqT[:, ck * wck:(ck + 1) * wck])

    # pre-warm the ACT exp table during phase A
    misc_pool = ctx.enter_context(tc.tile_pool(name="misc", bufs=1))
    warm = misc_pool.tile([128, 8], F32, tag="warm")
    nc.vector.memset(warm[:, 0:4], 0.0)
    nc.scalar.activation(warm[:, 4:8], warm[:, 0:4],
                         mybir.ActivationFunctionType.Exp)

    # ---------------- resident projections ----------------
    proj_pool = ctx.enter_context(tc.tile_pool(name="proj", bufs=1))
    qh_sb = [proj_pool.tile([128, HPC * S], BF16, tag=f"qh{b}", name=f"qh{b}")
             for b in range(B)]
    kh_sb = [proj_pool.tile([128, HPC * S], BF16, tag=f"kh{b}", name=f"kh{b}")
             for b in range(B)]
    vh_sb = [proj_pool.tile([128, NKT * DPC], BF16, tag=f"vh{b}", name=f"vh{b}")
             for b in range(B)]

    # ---------------- pools ----------------
    xcol_pool = ctx.enter_context(tc.tile_pool(name="xcol", bufs=2))
    es_pool = ctx.enter_context(tc.tile_pool(name="es", bufs=3))
    dts_pool = ctx.enter_context(tc.tile_pool(name="dts", bufs=10))
    acc_pool = ctx.enter_context(tc.tile_pool(name="acc", bufs=2))
    rb_pool = ctx.enter_context(tc.tile_pool(name="rb", bufs=2))
    ao_pool = ctx.enter_context(tc.tile_pool(name="ao", bufs=2))
    wo_pool = ctx.enter_context(tc.tile_pool(name="wo", bufs=2))
    osb_pool = ctx.enter_context(tc.tile_pool(name="osb", bufs=2))
    cc_pool = ctx.enter_context(tc.tile_pool(name="cc", bufs=B))

    ps_sc = ctx.enter_context(tc.tile_pool(name="psc", bufs=2, space="PSUM"))
    ps_pv = ctx.enter_context(tc.tile_pool(name="pspv", bufs=2, space="PSUM"))
    ps_a = ctx.enter_context(tc.tile_pool(name="psa", bufs=2, space="PSUM"))

    # ---------------- phase A chunks ----------------
    def gen_a_chunks(b):
        """K(8), V(16), Q(8) chunks for batch b. Each chunk = one 16-matmul
        accumulation group into one PSUM bank + a copy out."""
        chunks = []
        for kind, srcT, w_sb, dst in (
                ("k", kT[b], wk_sb, kh_sb[b]),
                ("v", vT[b], wv_sb, vh_sb[b]),
                ("q", qT[b], wq_sb, qh_sb[b])):
            for sb in range(NSB):
                holder = {}
                wfn = None
                if b == 0 and sb == 0:
                    wfn = emit_wv_dma if kind == "v" else (
                        emit_wq_dma if kind == "q" else None)

                def dma_fn(holder=holder, srcT=srcT, sb=sb, wfn=wfn):
                    if wfn is not None:
                        wfn()
                    t = xcol_pool.tile([128, NHT * QBLK], BF16, tag="xcol", name="xcol")
                    half = NHT * QBLK // 2
                    for h in range(2):
                        nc.sync.dma_start(
                            out=t[:, h * half:(h + 1) * half],
                            in_=srcT[sb * 128:(sb + 1) * 128,
                                     h * half:(h + 1) * half])
                    holder["t"] = t

                if kind in ("k", "q"):
                    for dt in range(HPC):
                        def comp(dep=None, holder=holder, w_sb=w_sb, dst=dst,
                                 sb=sb, dt=dt):
                            x = holder["t"]
                            ps = ps_a.tile([128, QBLK], F32, tag="pa")
                            for ht in range(NHT):
                                mm = nc.tensor.matmul(
                                    ps[:],
                                    lhsT=w_sb[:, ht * DPC + dt * 128:
                                              ht * DPC + (dt + 1) * 128],
                                    rhs=x[:, ht * QBLK:(ht + 1) * QBLK],
                                    start=(ht == 0), stop=(ht == NHT - 1))
                                if ht == 0 and dep is not None:
                                    tile.add_dep_helper(mm.ins, dep.ins,
                                                        reason="defer")
                            nc.vector.tensor_copy(
                                dst[:, dt * S + sb * QBLK:
                                    dt * S + (sb + 1) * QBLK], ps[:])
                        chunks.append((dma_fn if dt == 0 else None, comp))
                else:
                    for st in range(QBLK // 128):
                        def comp(dep=None, holder=holder, w_sb=w_sb, dst=dst,
                                 sb=sb, st=st):
                            x = holder["t"]
                            ps = ps_a.tile([128, DPC], F32, tag="pa")
                            for ht in range(NHT):
                                mm = nc.tensor.matmul(
                                    ps[:],
                                    lhsT=x[:, ht * QBLK + st * 128:
                                           ht * QBLK + (st + 1) * 128],
                                    rhs=w_sb[:, ht * DPC:(ht + 1) * DPC],
                                    start=(ht == 0), stop=(ht == NHT - 1))
                                if ht == 0 and dep is not None:
                                    tile.add_dep_helper(mm.ins, dep.ins,
                                                        reason="defer")
                            kt = sb * (QBLK // 128) + st
                            nc.vector.tensor_copy(
                                dst[:, kt * DPC:(kt + 1) * DPC], ps[:])
                        chunks.append((dma_fn if st == 0 else None, comp))
        return chunks

    # ---------------- phase B ----------------
    def emit_b_group(b, qb, l, g, state):
        sc = ps_sc.tile([128, GK * QBLK], F32, tag="sc")
        if g == 0:
            state["pv"] = ps_pv.tile([128, QBLK], F32, tag="pv", name="pv")
        pv = state["pv"]
        for j in range(GK):
            kt = g * GK + j
            nc.tensor.matmul(
                sc[:, j * QBLK:(j + 1) * QBLK],
                lhsT=kh_sb[b][:, l * S + kt * 128:l * S + (kt + 1) * 128],
                rhs=qh_sb[b][:, l * S + qb * QBLK:l * S + (qb + 1) * QBLK],
                start=True, stop=True)
        es = es_pool.tile([128, GK * QBLK], BF16, tag="es")
        exp_inst = nc.scalar.activation(es[:], sc[:],
                                        mybir.ActivationFunctionType.Exp,
                                        scale=scale)
        for j in range(GK):
            kt = g * GK + j
            nc.tensor.matmul(
                pv[:],
                lhsT=vh_sb[b][:, kt * DPC + l * 128:kt * DPC + (l + 1) * 128],
                rhs=es[:, j * QBLK:(j + 1) * QBLK],
                start=(kt == 0), stop=(kt == NKT - 1))
        # denominator: fold this group's 2 es tiles, then log-depth pair
        # tree across groups (all on DVE; Pool is too slow for this)
        sg = dts_pool.tile([128, QBLK], BF16, tag="dts")
        nc.vector.tensor_add(sg[:], es[:, 0:QBLK], es[:, QBLK:2 * QBLK])
        pend = state.setdefault("pend", {})
        node, level = sg, 0
        while level in pend:
            nxt = dts_pool.tile([128, QBLK], BF16, tag="dts", name="dtn")
            nc.vector.tensor_add(nxt[:], pend.pop(level)[:], node[:])
            node, level = nxt, level + 1
        pend[level] = node
        return exp_inst

    def emit_b_fin(b, qb, l, state):
        pend = state["pend"]
        nodes = [pend[k] for k in sorted(pend)]
        node = nodes[0]
        for other in nodes[1:]:
            nxt = dts_pool.tile([128, QBLK], BF16, tag="dts", name="dtf")
            nc.vector.tensor_add(nxt[:], node[:], other[:])
            node = nxt
        acc = acc_pool.tile([128, QBLK], F32, tag="acc")
        nc.vector.tensor_copy(acc[:], node[:])
        rb = rb_pool.tile([128, QBLK], F32, tag="rb")
        nc.gpsimd.partition_all_reduce(rb[:], acc[:], channels=128,
                                       reduce_op=bass_isa.ReduceOp.add)
        nc.vector.reciprocal_approx_fast(rb[:], rb[:])
        ao = ao_pool.tile([128, QBLK], BF16, tag="ao")
        nc.vector.tensor_mul(ao[:], state["pv"][:], rb[:])
        # scatter into a2a_in[b]: owner m gets rows [m*DPC+l*128, +128)
        q0 = qb * QBLK
        while q0 < (qb + 1) * QBLK:
            m = q0 // SCB
            cend = min((qb + 1) * QBLK, (m + 1) * SCB)
            nc.gpsimd.dma_start(
                out=a2a_in[b][m * DPC + l * 128:m * DPC + (l + 1) * 128,
                              q0 - m * SCB:cend - m * SCB],
                in_=ao[:, q0 - qb * QBLK:cend - qb * QBLK])
            q0 = cend

    def gen_b_units(b):
        """Flat list of emit-callables: 4 groups + finalize per (qb, l)."""
        units = []
        for qb in range(NQB):
            for l in range(HPC):
                state = {}
                for g in range(NG):
                    units.append(
                        lambda b=b, qb=qb, l=l, g=g, state=state:
                        emit_b_group(b, qb, l, g, state))
                units.append(
                    lambda b=b, qb=qb, l=l, state=state:
                    emit_b_fin(b, qb, l, state))
        return units

    # ---------------- collectives + gather ----------------
    cc_tiles = [None, None]
    colls = [None, None]

    def emit_a2a(b):
        colls[b] = nc.gpsimd.collective_compute(
            "AllToAll", mybir.AluOpType.bypass,
            replica_groups=[list(range(N_CORES))],
            ins=[a2a_in[b][:, :]], outs=[a2a_out[b][:, :]])

    def emit_cc_gather(b):
        cc_sb = cc_pool.tile([128, NHT * SCB], BF16, tag="cc", name=f"cc{b}")
        dma = nc.sync.dma_start(
            out=cc_sb[:].rearrange("p (t s) -> p t s", t=NHT),
            in_=a2a_out[b][:, :].rearrange("(t p) s -> p t s", p=128))
        tile.add_dep_helper(dma.ins, colls[b].ins,
                            reason="a2a_out after collective")
        cc_tiles[b] = cc_sb

    # ---------------- phase D chunks ----------------
    NOB = HIDDEN // QBLK            # 4 output col blocks
    NST = SCB // 128                # 2 row tiles

    def gen_d_chunks(b):
        chunks = []
        for ot in range(NOB):
            holder = {}

            def dma_fn(holder=holder, ot=ot):
                blk = wo_pool.tile([128, NHT * QBLK], BF16, tag="wo", name="woblk")
                half = NHT * QBLK // 2
                for h in range(2):
                    nc.sync.dma_start(
                        out=blk[:, h * half:(h + 1) * half],
                        in_=woT[ot * 128:(ot + 1) * 128,
                                h * half:(h + 1) * half])
                holder["t"] = blk

            for st in range(NST):
                def comp(dep=None, holder=holder, b=b, st=st, ot=ot):
                    blk = holder["t"]
                    cc_sb = cc_tiles[b]
                    ps = ps_a.tile([128, QBLK], F32, tag="pa")
                    for t in range(NHT):
                        mm = nc.tensor.matmul(
                            ps[:],
                            lhsT=cc_sb[:, t * SCB + st * 128:
                                       t * SCB + (st + 1) * 128],
                            rhs=blk[:, t * QBLK:(t + 1) * QBLK],
                            start=(t == 0), stop=(t == NHT - 1))
                        if t == 0 and dep is not None:
                            tile.add_dep_helper(mm.ins, dep.ins,
                                                reason="defer D after B")
                    osb = osb_pool.tile([128, QBLK], BF16, tag="osb")
                    nc.vector.tensor_copy(osb[:], ps[:])
                    nc.sync.dma_start(
                        out=out[b * SCB + st * 128:b * SCB + (st + 1) * 128,
                                ot * QBLK:(ot + 1) * QBLK],
                        in_=osb[:])
                chunks.append((dma_fn if st == 0 else None, comp))
        return chunks

    # ================= master emission order =================
    # A0 dense
    ChunkStream(gen_a_chunks(0)).drain()

    # B0 (64 groups + 8 fins) interleaved with A1. The last 4 Q1 chunks
    # (qb2/qb3) are held back: they become PE filler for B1's ACT-bound
    # stretch, and B1 only needs those qh blocks in its second half.
    a1_all = gen_a_chunks(1)
    a1 = ChunkStream(a1_all[:-4])
    a1_rest = ChunkStream(a1_all[-4:])
    for i, unit in enumerate(gen_b_units(0)):
        unit()
        if i % 2 == 0:
            a1.emit_one()
    a1.drain()
    emit_a2a(0)

    # B1 with the held-back Q1 chunks as early filler; cc0 gather once
    # coll0 has landed.  D0 is NOT interleaved here: its matmuls depend on
    # the collective, whose latency the scheduler underestimates — any D0
    # placed inside B1 head-blocks the in-order PE queue on the real a2a.
    b1_units = gen_b_units(1)
    anchor = None
    for i, unit in enumerate(b1_units):
        r = unit()
        if r is not None:
            anchor = r
        if i == 30:
            emit_cc_gather(0)
        if i in (4, 10, 16, 22):
            a1_rest.emit_one()
    emit_a2a(1)

    # all of D0 runs while a2a1 is in flight; every chunk is dep-anchored
    # on B1's last exp so the scheduler (whose collective model is
    # optimistic) cannot hoist any D0 matmul ahead of ready B1 work.
    d0 = ChunkStream(gen_d_chunks(0))
    while d0.emit_one(dep=anchor):
        pass
    emit_cc_gather(1)
    ChunkStream(gen_d_chunks(1)).drain()


def build_nc(S: int):
    nc = bacc.Bacc("TRN2", target_bir_lowering=False, debug=False,
                   enable_asserts=False, num_devices=N_CORES)
    SCB = S // N_CORES
    aps = {
        "qT": [nc.dram_tensor(f"qT{b}", [(S // 512) * 128, NHT * 512], BF16,
                              kind="ExternalInput").ap() for b in range(B)],
        "kT": [nc.dram_tensor(f"kT{b}", [(S // 512) * 128, NHT * 512], BF16,
                              kind="ExternalInput").ap() for b in range(B)],
        "vT": [nc.dram_tensor(f"vT{b}", [(S // 512) * 128, NHT * 512], BF16,
                              kind="ExternalInput").ap() for b in range(B)],
        "wqT": nc.dram_tensor("wqT", [128, NHT * DPC], BF16,
                              kind="ExternalInput").ap(),
        "wkT": nc.dram_tensor("wkT", [128, NHT * DPC], BF16,
                              kind="ExternalInput").ap(),
        "wvT": nc.dram_tensor("wvT", [128, NHT * DPC], BF16,
                              kind="ExternalInput").ap(),
        "woT": nc.dram_tensor("woT", [(HIDDEN // 512) * 128, NHT * 512],
                              BF16, kind="ExternalInput").ap(),
        "out": nc.dram_tensor("out", [B * SCB, HIDDEN], BF16,
                              kind="ExternalOutput").ap(),
        "a2a_in": [nc.dram_tensor(f"a2a_in{b}", [N_CORES * DPC, SCB],
                                  BF16).ap() for b in range(B)],
        "a2a_out": [nc.dram_tensor(f"a2a_out{b}", [N_CORES * DPC, SCB],
                                   BF16).ap() for b in range(B)],
    }
    with tile.TileContext(nc) as tc:
        with ExitStack() as ctx:
            _mha_kernel(ctx, tc, aps, S)
    nc.compile()
    return nc


_NC_CACHE: dict = {}


def _tile_weight(w_slice_T):
    """[H, D] -> [128, (H//128)*D] with 128-row tiles laid out consecutively."""
    H, D = w_slice_T.shape
    return np.ascontiguousarray(
        w_slice_T.reshape(H // 128, 128, D).transpose(1, 0, 2).reshape(
            128, (H // 128) * D))


def _swizzle_x(xT):
    """[H, S] -> [(S//512)*128, (H//128)*512]: row (sb*128+p), col
    (ht*512+j) = xT[ht*128+p, sb*512+j].  Each DMA block is then a 2D
    contiguous-per-partition transfer (16 KB lines)."""
    H, S = xT.shape
    nht, nsb = H // 128, S // 512
    return np.ascontiguousarray(
        xT.reshape(nht, 128, nsb, 512).transpose(2, 1, 0, 3).reshape(
            nsb * 128, nht * 512))


def make_in_maps(q, k, v, w_q, w_k, w_v, w_o):
    """Host-side shard/cast. Returns per-core input dicts."""
    qT = [_swizzle_x(q[b].T.astype(NPBF16)) for b in range(B)]
    kT = [_swizzle_x(k[b].T.astype(NPBF16)) for b in range(B)]
    vT = [_swizzle_x(v[b].T.astype(NPBF16)) for b in range(B)]
    woT = _swizzle_x(w_o.astype(NPBF16).T)
    in_maps = []
    for c in range(N_CORES):
        d0 = c * DPC
        m = {}
        for b in range(B):
            m[f"qT{b}"] = qT[b]
            m[f"kT{b}"] = kT[b]
            m[f"vT{b}"] = vT[b]
        m["wqT"] = _tile_weight(
            np.ascontiguousarray(w_q[d0:d0 + DPC, :].T).astype(NPBF16))
        m["wkT"] = _tile_weight(
            np.ascontiguousarray(w_k[d0:d0 + DPC, :].T).astype(NPBF16))
        m["wvT"] = _tile_weight(
            np.ascontiguousarray(w_v[d0:d0 + DPC, :].T).astype(NPBF16))
        m["woT"] = woT
        in_maps.append(m)
    return in_maps


def kernel(q, k, v, mask, w_q, w_k, w_v, w_o, _trace=False):
    q = np.asarray(q, np.float32)
    k = np.asarray(k, np.float32)
    v = np.asarray(v, np.float32)
    mask = np.asarray(mask)
    w_q = np.asarray(w_q, np.float32)
    w_k = np.asarray(w_k, np.float32)
    w_v = np.asarray(w_v, np.float32)
    w_o = np.asarray(w_o, np.float32)
    S = q.shape[1]

    if not np.all(mask != 0):
        # General-mask fallback (never hit for the eval problem: mask is all
        # ones).  Computed on host for correctness.
        return _numpy_reference(q, k, v, mask, w_q, w_k, w_v, w_o)

    if S not in _NC_CACHE:
        _NC_CACHE[S] = build_nc(S)
    nc = _NC_CACHE[S]

    in_maps = make_in_maps(q, k, v, w_q, w_k, w_v, w_o)
    res = run_bass_kernel_spmd(nc, in_maps, core_ids=list(range(N_CORES)),
                               trace=_trace)

    SCB = S // N_CORES
    out = np.empty((B, S, HIDDEN), np.float32)
    for c in range(N_CORES):
        for b in range(B):
            out[b, c * SCB:(c + 1) * SCB, :] = \
                res.results[c]["out"][b * SCB:(b + 1) * SCB, :].astype(
                    np.float32)
    if _trace:
        return out, res
    return out


def _numpy_reference(q, k, v, mask, w_q, w_k, w_v, w_o):
    Bn, S, H = q.shape
    dk = H // HEADS

    def split_heads(x, w):
        y = x @ w.T
        return y.reshape(Bn, S, HEADS, dk).transpose(0, 2, 1, 3)

    qh = split_heads(q, w_q)
    kh = split_heads(k, w_k)
    vh = split_heads(v, w_v)
    s = np.einsum("bhqd,bhkd->bhqk", qh, kh) / np.sqrt(np.float32(dk))
    s = np.where(mask[:, None, :, :] == 0, np.float32(-1e9), s)
    s = s - s.max(-1, keepdims=True)
    e = np.exp(s)
    a = e / e.sum(-1, keepdims=True)
    o = np.einsum("bhqk,bhkd->bhqd", a, vh)
    o = o.transpose(0, 2, 1, 3).reshape(Bn, S, H)
    return (o @ w_o.T).astype(np.float32)


# revision 9
# speedup vs baseline: 1.1411x; 1.1411x over previous
"""Multi-head attention forward on 8 Trainium2 NeuronCores (Bass/Tile).

Problem: B=2, S=2048, HIDDEN=2048, HEADS=16, D_K=128, fp32 I/O,
mask all-ones (eval). torch-Linear convention: y = x @ W.T.

Sharding (head + output-row parallel, two AllToAlls, no all-reduce):
  core c (0..7) owns heads {2c, 2c+1} for BOTH batches; after attention an
  8-rank AllToAll re-shards from head-split to row-split for the W_o GEMM.

v2 — single continuous PE stream (HAM-warm, no phase barriers):
  - A (projections): ht-inner accumulation chunks (16 matmuls into ONE psum
    bank), streaming x column-blocks. Q,K projected into [d, s] layout
    (W-stationary, N=512); V projected directly into the natural [s, d]
    layout (x-stationary, N=256) - no PE transposes at all.
  - B (attention): per (qb,l): 4 groups of 4 kt; scores matmuls fill a
    4-bank [128, 2048] PSUM tile; ONE big exp on ACT per group; PV
    accumulates in [d, q]. Softmax denominators: DVE pair-tree + Pool
    cross-group adds + one GPSIMD partition_all_reduce (PE never touches
    softmax bookkeeping).
  - Cross-phase interleave in emission (= Tile priority) order:
      A0 dense | B0 x A1 | a2a0 | B1 x D0(half) | a2a1 | D0(rest), D1
    so the PE always has independent matmuls to run while ACT does exp and
    while collectives are in flight.
  - Output written bf16 (halves the tail DMA); host upcasts to f32.
Host side: pre-transpose/cast inputs to bf16, slice weights per core,
scatter-gather the per-core [512, 2048] chunks into the full output.
"""

import math
from contextlib import ExitStack

import ml_dtypes
import numpy as np

import concourse.bass as bass
import concourse.bass_isa as bass_isa
import concourse.tile as tile
from concourse import bacc, mybir
from concourse.bass_utils import run_bass_kernel_spmd

BF16 = mybir.dt.bfloat16
F32 = mybir.dt.float32
NPBF16 = ml_dtypes.bfloat16

HIDDEN = 2048
HEADS = 16
D_K = 128
B = 2
N_CORES = 8
HPC = HEADS // N_CORES          # heads per core (2)
DPC = HPC * D_K                 # concat cols per core (256)
NHT = HIDDEN // 128             # 16 hidden-dim 128-tiles


class ChunkStream:
    """Ordered chunks of (dma_fn|None, compute_fn); emits with prefetch."""

    def __init__(self, chunks, prefetch=2):
        self.chunks = list(chunks)
        self.ci = 0     # next compute
        self.di = 0     # next dma
        self.prefetch = prefetch

    def _run_dmas(self):
        # issue dma for chunks up to `prefetch` compute-chunks ahead
        while self.di < len(self.chunks) and self.di < self.ci + self.prefetch:
            dma_fn, _ = self.chunks[self.di]
            if dma_fn is not None:
                dma_fn()
            self.di += 1

    def emit_one(self, dep=None):
        if self.ci >= len(self.chunks):
            return False
        self._run_dmas()
        self.chunks[self.ci][1](dep=dep)
        self.ci += 1
        self._run_dmas()
        return True

    def drain(self):
        while self.emit_one():
            pass


def _mha_kernel(ctx: ExitStack, tc: tile.TileContext, aps: dict, S: int):
    nc = tc.nc
    NKT = S // 128                   # seq 128-tiles (16)
    QBLK = 512
    NQB = S // QBLK                  # 4
    NSB = S // QBLK                  # x col-blocks per tensor (4)
    GK = 2                           # kt per exp-group
    NG = NKT // GK                   # groups per (qb,l) (8)
    SCB = S // N_CORES               # per-batch output rows per core (256)
    scale = 1.0 / math.sqrt(D_K)

    qT, kT, vT = aps["qT"], aps["kT"], aps["vT"]   # per batch [HIDDEN, S]
    wqT, wkT, wvT = aps["wqT"], aps["wkT"], aps["wvT"]  # [128, NHT*DPC]
    woT = aps["woT"]                                # [128, NHT*HIDDEN]
    out = aps["out"]                                # [B*SCB, HIDDEN] bf16
    a2a_in = aps["a2a_in"]                          # per batch [8*DPC, SCB]
    a2a_out = aps["a2a_out"]                        # per batch [8*DPC, SCB]

    # ---------------- resident weights ----------------
    w_pool = ctx.enter_context(tc.tile_pool(name="wqkv", bufs=1))
    wq_sb = w_pool.tile([128, NHT * DPC], BF16, tag="wq")
    wk_sb = w_pool.tile([128, NHT * DPC], BF16, tag="wk")
    wv_sb = w_pool.tile([128, NHT * DPC], BF16, tag="wv")
    wck = NHT * DPC // 4
    # K is projected first: its weight chunks go out first.  wv/wq are
    # DMAed lazily from the first V/Q chunk so the first K matmul's
    # inputs hit the DMA queues with no competition.
    for ck in range(4):
        nc.sync.dma_start(out=wk_sb[:, ck * wck:(ck + 1) * wck],
                          in_=wkT[:, ck * wck:(ck + 1) * wck])
    def emit_wv_dma():
        for ck in range(4):
            nc.sync.dma_start(out=wv_sb[:, ck * wck:(ck + 1) * wck],
                              in_=wvT[:, ck * wck:(ck + 1) * wck])

    def emit_wq_dma():
        for ck in range(4):
            nc.sync.dma_start(out=wq_sb[:, ck * wck:(ck + 1) * wck],
                              in_=wqT[:, ck * wck:(ck + 1) * wck])

    # pre-warm the ACT exp table during phase A
    misc_pool = ctx.enter_context(tc.tile_pool(name="misc", bufs=1))
    warm = misc_pool.tile([128, 8], F32, tag="warm")
    nc.vector.memset(warm[:, 0:4], 0.0)
    nc.scalar.activation(warm[:, 4:8], warm[:, 0:4],
                         mybir.ActivationFunctionType.Exp)

    # ---------------- resident projections ----------------
    proj_pool = ctx.enter_context(tc.tile_pool(name="proj", bufs=1))
    qh_sb = [proj_pool.tile([128, HPC * S], BF16, tag=f"qh{b}", name=f"qh{b}")
             for b in range(B)]
    kh_sb = [proj_pool.tile([128, HPC * S], BF16, tag=f"kh{b}", name=f"kh{b}")
             for b in range(B)]
    vh_sb = [proj_pool.tile([128, NKT * DPC], BF16, tag=f"vh{b}", name=f"vh{b}")
             for b in range(B)]

    # ---------------- pools ----------------
    xcol_pool = ctx.enter_context(tc.tile_pool(name="xcol", bufs=2))
    es_pool = ctx.enter_context(tc.tile_pool(name="es", bufs=3))
    dts_pool = ctx.enter_context(tc.tile_pool(name="dts", bufs=10))
    acc_pool = ctx.enter_context(tc.tile_pool(name="acc", bufs=2))
    rb_pool = ctx.enter_context(tc.tile_pool(name="rb", bufs=2))
    ao_pool = ctx.enter_context(tc.tile_pool(name="ao", bufs=2))
    wo_pool = ctx.enter_context(tc.tile_pool(name="wo", bufs=2))
    osb_pool = ctx.enter_context(tc.tile_pool(name="osb", bufs=2))
    cc_pool = ctx.enter_context(tc.tile_pool(name="cc", bufs=B))

    ps_sc = ctx.enter_context(tc.tile_pool(name="psc", bufs=2, space="PSUM"))
    ps_pv = ctx.enter_context(tc.tile_pool(name="pspv", bufs=2, space="PSUM"))
    ps_a = ctx.enter_context(tc.tile_pool(name="psa", bufs=2, space="PSUM"))

    # ---------------- phase A chunks ----------------
    def gen_a_chunks(b):
        """K(8), V(16), Q(8) chunks for batch b. Each chunk = one 16-matmul
        accumulation group into one PSUM bank + a copy out."""
        chunks = []
        for kind, srcT, w_sb, dst in (
                ("k", kT[b], wk_sb, kh_sb[b]),
                ("v", vT[b], wv_sb, vh_sb[b]),
                ("q", qT[b], wq_sb, qh_sb[b])):
            for sb in range(NSB):
                holder = {}
                wfn = None
                if b == 0 and sb == 0:
                    wfn = emit_wv_dma if kind == "v" else (
                        emit_wq_dma if kind == "q" else None)

                def dma_fn(holder=holder, srcT=srcT, sb=sb, wfn=wfn):
                    if wfn is not None:
                        wfn()
                    t = xcol_pool.tile([128, NHT * QBLK], BF16, tag="xcol", name="xcol")
                    half = NHT * QBLK // 2
                    for h in range(2):
                        nc.sync.dma_start(
                            out=t[:, h * half:(h + 1) * half],
                            in_=srcT[sb * 128:(sb + 1) * 128,
                                     h * half:(h + 1) * half])
                    holder["t"] = t

                if kind in ("k", "q"):
                    for dt in range(HPC):
                        def comp(dep=None, holder=holder, w_sb=w_sb, dst=dst,
                                 sb=sb, dt=dt):
                            x = holder["t"]
                            ps = ps_a.tile([128, QBLK], F32, tag="pa")
                            for ht in range(NHT):
                                mm = nc.tensor.matmul(
                                    ps[:],
                                    lhsT=w_sb[:, ht * DPC + dt * 128:
                                              ht * DPC + (dt + 1) * 128],
                                    rhs=x[:, ht * QBLK:(ht + 1) * QBLK],
                                    start=(ht == 0), stop=(ht == NHT - 1))
                                if ht == 0 and dep is not None:
                                    tile.add_dep_helper(mm.ins, dep.ins,
                                                        reason="defer")
                            nc.vector.tensor_copy(
                                dst[:, dt * S + sb * QBLK:
                                    dt * S + (sb + 1) * QBLK], ps[:])
                        chunks.append((dma_fn if dt == 0 else None, comp))
                else:
                    for st in range(QBLK // 128):
                        def comp(dep=None, holder=holder, w_sb=w_sb, dst=dst,
                                 sb=sb, st=st):
                            x = holder["t"]
                            ps = ps_a.tile([128, DPC], F32, tag="pa")
                            for ht in range(NHT):
                                mm = nc.tensor.matmul(
                                    ps[:],
                                    lhsT=x[:, ht * QBLK + st * 128:
                                           ht * QBLK + (st + 1) * 128],
                                    rhs=w_sb[:, ht * DPC:(ht + 1) * DPC],
                                    start=(ht == 0), stop=(ht == NHT - 1))
                                if ht == 0 and dep is not None:
                                    tile.add_dep_helper(mm.ins, dep.ins,
                                                        reason="defer")
                            kt = sb * (QBLK // 128) + st
                            nc.vector.tensor_copy(
                                dst[:, kt * DPC:(kt + 1) * DPC], ps[:])
                        chunks.append((dma_fn if st == 0 else None, comp))
        return chunks

    # ---------------- phase B ----------------
    def emit_b_group(b, qb, l, g, state):
        sc = ps_sc.tile([128, GK * QBLK], F32, tag="sc")
        if g == 0:
            state["pv"] = ps_pv.tile([128, QBLK], F32, tag="pv", name="pv")
        pv = state["pv"]
        for j in range(GK):
            kt = g * GK + j
            nc.tensor.matmul(
                sc[:, j * QBLK:(j + 1) * QBLK],
                lhsT=kh_sb[b][:, l * S + kt * 128:l * S + (kt + 1) * 128],
                rhs=qh_sb[b][:, l * S + qb * QBLK:l * S + (qb + 1) * QBLK],
                start=True, stop=True)
        es = es_pool.tile([128, GK * QBLK], BF16, tag="es")
        exp_inst = nc.scalar.activation(es[:], sc[:],
                                        mybir.ActivationFunctionType.Exp,
                                        scale=scale)
        for j in range(GK):
            kt = g * GK + j
            nc.tensor.matmul(
                pv[:],
                lhsT=vh_sb[b][:, kt * DPC + l * 128:kt * DPC + (l + 1) * 128],
                rhs=es[:, j * QBLK:(j + 1) * QBLK],
                start=(kt == 0), stop=(kt == NKT - 1))
        # denominator: fold this group's 2 es tiles, then log-depth pair
        # tree across groups (all on DVE; Pool is too slow for this)
        sg = dts_pool.tile([128, QBLK], BF16, tag="dts")
        nc.vector.tensor_add(sg[:], es[:, 0:QBLK], es[:, QBLK:2 * QBLK])
        pend = state.setdefault("pend", {})
        node, level = sg, 0
        while level in pend:
            nxt = dts_pool.tile([128, QBLK], BF16, tag="dts", name="dtn")
            nc.vector.tensor_add(nxt[:], pend.pop(level)[:], node[:])
            node, level = nxt, level + 1
        pend[level] = node
        return exp_inst

    def emit_b_fin(b, qb, l, state):
        pend = state["pend"]
        nodes = [pend[k] for k in sorted(pend)]
        node = nodes[0]
        for other in nodes[1:]:
            nxt = dts_pool.tile([128, QBLK], BF16, tag="dts", name="dtf")
            nc.vector.tensor_add(nxt[:], node[:], other[:])
            node = nxt
        acc = acc_pool.tile([128, QBLK], F32, tag="acc")
        nc.vector.tensor_copy(acc[:], node[:])
        rb = rb_pool.tile([128, QBLK], F32, tag="rb")
        nc.gpsimd.partition_all_reduce(rb[:], acc[:], channels=128,
                                       reduce_op=bass_isa.ReduceOp.add)
        nc.vector.reciprocal_approx_fast(rb[:], rb[:])
        ao = ao_pool.tile([128, QBLK], BF16, tag="ao")
        nc.vector.tensor_mul(ao[:], state["pv"][:], rb[:])
        # scatter into a2a_in[b]: owner m gets rows [m*DPC+l*128, +128)
        q0 = qb * QBLK
        while q0 < (qb + 1) * QBLK:
            m = q0 // SCB
            cend = min((qb + 1) * QBLK, (m + 1) * SCB)
            nc.gpsimd.dma_start(
                out=a2a_in[b][m * DPC + l * 128:m * DPC + (l + 1) * 128,
                              q0 - m * SCB:cend - m * SCB],
                in_=ao[:, q0 - qb * QBLK:cend - qb * QBLK])
            q0 = cend

    def gen_b_units(b):
        """Flat list of emit-callables: 4 groups + finalize per (qb, l)."""
        units = []
        for qb in range(NQB):
            for l in range(HPC):
                state = {}
                for g in range(NG):
                    units.append(
                        lambda b=b, qb=qb, l=l, g=g, state=state:
                        emit_b_group(b, qb, l, g, state))
                units.append(
                    lambda b=b, qb=qb, l=l, state=state:
                    emit_b_fin(b, qb, l, state))
        return units

    # ---------------- collectives + gather ----------------
    cc_tiles = [None, None]
    colls = [None, None]

    def emit_a2a(b):
        colls[b] = nc.gpsimd.collective_compute(
            "AllToAll", mybir.AluOpType.bypass,
            replica_groups=[list(range(N_CORES))],
            ins=[a2a_in[b][:, :]], outs=[a2a_out[b][:, :]])

    def emit_cc_gather(b):
        cc_sb = cc_pool.tile([128, NHT * SCB], BF16, tag="cc", name=f"cc{b}")
        dma = nc.sync.dma_start(
            out=cc_sb[:].rearrange("p (t s) -> p t s", t=NHT),
            in_=a2a_out[b][:, :].rearrange("(t p) s -> p t s", p=128))
        tile.add_dep_helper(dma.ins, colls[b].ins,
                            reason="a2a_out after collective")
        cc_tiles[b] = cc_sb

    # ---------------- phase D chunks ----------------
    d_last_mm = [None]
    NOB = HIDDEN // QBLK            # 4 output col blocks
    NST = SCB // 128                # 2 row tiles

    def gen_d_chunks(b):
        chunks = []
        for ot in range(NOB):
            holder = {}

            def dma_fn(holder=holder, ot=ot):
                blk = wo_pool.tile([128, NHT * QBLK], BF16, tag="wo", name="woblk")
                half = NHT * QBLK // 2
                for h in range(2):
                    nc.sync.dma_start(
                        out=blk[:, h * half:(h + 1) * half],
                        in_=woT[ot * 128:(ot + 1) * 128,
                                h * half:(h + 1) * half])
                holder["t"] = blk

            for st in range(NST):
                def comp(dep=None, holder=holder, b=b, st=st, ot=ot):
                    blk = holder["t"]
                    cc_sb = cc_tiles[b]
                    ps = ps_a.tile([128, QBLK], F32, tag="pa")
                    first_mm = None
                    for t in range(NHT):
                        mm = nc.tensor.matmul(
                            ps[:],
                            lhsT=cc_sb[:, t * SCB + st * 128:
                                       t * SCB + (st + 1) * 128],
                            rhs=blk[:, t * QBLK:(t + 1) * QBLK],
                            start=(t == 0), stop=(t == NHT - 1))
                        if t == 0:
                            first_mm = mm
                            if dep is not None:
                                tile.add_dep_helper(mm.ins, dep.ins,
                                                    reason="defer D after B")
                    d_last_mm[0] = first_mm
                    osb = osb_pool.tile([128, QBLK], BF16, tag="osb")
                    nc.vector.tensor_copy(osb[:], ps[:])
                    nc.sync.dma_start(
                        out=out[b * SCB + st * 128:b * SCB + (st + 1) * 128,
                                ot * QBLK:(ot + 1) * QBLK],
                        in_=osb[:])
                chunks.append((dma_fn if st == 0 else None, comp))
        return chunks

    # ================= master emission order =================
    # A0 dense
    ChunkStream(gen_a_chunks(0)).drain()

    # B0 (64 groups + 8 fins) interleaved with A1. The last 4 Q1 chunks
    # (qb2/qb3) are held back: they become PE filler for B1's ACT-bound
    # stretch, and B1 only needs those qh blocks in its second half.
    a1_all = gen_a_chunks(1)
    a1 = ChunkStream(a1_all[:-4])
    a1_rest = ChunkStream(a1_all[-4:])
    for i, unit in enumerate(gen_b_units(0)):
        unit()
        if i % 2 == 0:
            a1.emit_one()
    a1.drain()
    emit_a2a(0)

    # B1 with the held-back Q1 chunks as early filler; cc0 gather once
    # coll0 has landed.  D0 is NOT interleaved here: its matmuls depend on
    # the collective, whose latency the scheduler underestimates — any D0
    # placed inside B1 head-blocks the in-order PE queue on the real a2a.
    b1_units = gen_b_units(1)
    anchor = None
    for i, unit in enumerate(b1_units):
        r = unit()
        if r is not None and i <= 48:
            anchor = r
        if i == 30:
            emit_cc_gather(0)
        if i in (4, 10, 16, 22):
            a1_rest.emit_one()
    emit_a2a(1)

    # D0 runs in B1's tail / the a2a1 flight.  Every chunk is dep-anchored
    # on a mid-late B1 exp: late enough that the real a2a0 has landed by
    # then, early enough that D0's sim-ready time stays clearly before
    # cc1's, so the scheduler (whose collective model is optimistic)
    # neither hoists D0 ahead of ready B1 work nor interleaves D1 first.
    d0 = ChunkStream(gen_d_chunks(0))
    while d0.emit_one(dep=anchor):
        pass
    emit_cc_gather(1)
    # pin D1 strictly after D0 in the PE queue
    d1 = ChunkStream(gen_d_chunks(1))
    while d1.emit_one(dep=d_last_mm[0]):
        pass


def build_nc(S: int):
    nc = bacc.Bacc("TRN2", target_bir_lowering=False, debug=False,
                   enable_asserts=False, num_devices=N_CORES)
    SCB = S // N_CORES
    aps = {
        "qT": [nc.dram_tensor(f"qT{b}", [(S // 512) * 128, NHT * 512], BF16,
                              kind="ExternalInput").ap() for b in range(B)],
        "kT": [nc.dram_tensor(f"kT{b}", [(S // 512) * 128, NHT * 512], BF16,
                              kind="ExternalInput").ap() for b in range(B)],
        "vT": [nc.dram_tensor(f"vT{b}", [(S // 512) * 128, NHT * 512], BF16,
                              kind="ExternalInput").ap() for b in range(B)],
        "wqT": nc.dram_tensor("wqT", [128, NHT * DPC], BF16,
                              kind="ExternalInput").ap(),
        "wkT": nc.dram_tensor("wkT", [128, NHT * DPC], BF16,
                              kind="ExternalInput").ap(),
        "wvT": nc.dram_tensor("wvT", [128, NHT * DPC], BF16,
                              kind="ExternalInput").ap(),
        "woT": nc.dram_tensor("woT", [(HIDDEN // 512) * 128, NHT * 512],
                              BF16, kind="ExternalInput").ap(),
        "out": nc.dram_tensor("out", [B * SCB, HIDDEN], BF16,
                              kind="ExternalOutput").ap(),
        "a2a_in": [nc.dram_tensor(f"a2a_in{b}", [N_CORES * DPC, SCB],
                                  BF16).ap() for b in range(B)],
        "a2a_out": [nc.dram_tensor(f"a2a_out{b}", [N_CORES * DPC, SCB],
                                   BF16).ap() for b in range(B)],
    }
    with tile.TileContext(nc) as tc:
        with ExitStack() as ctx:
            _mha_kernel(ctx, tc, aps, S)
    nc.compile()
    return nc


_NC_CACHE: dict = {}


def _tile_weight(w_slice_T):
    """[H, D] -> [128, (H//128)*D] with 128-row tiles laid out consecutively."""
    H, D = w_slice_T.shape
    return np.ascontiguousarray(
        w_slice_T.reshape(H // 128, 128, D).transpose(1, 0, 2).reshape(
            128, (H // 128) * D))


def _swizzle_x(xT):
    """[H, S] -> [(S//512)*128, (H//128)*512]: row (sb*128+p), col
    (ht*512+j) = xT[ht*128+p, sb*512+j].  Each DMA block is then a 2D
    contiguous-per-partition transfer (16 KB lines)."""
    H, S = xT.shape
    nht, nsb = H // 128, S // 512
    return np.ascontiguousarray(
        xT.reshape(nht, 128, nsb, 512).transpose(2, 1, 0, 3).reshape(
            nsb * 128, nht * 512))


def make_in_maps(q, k, v, w_q, w_k, w_v, w_o):
    """Host-side shard/cast. Returns per-core input dicts."""
    qT = [_swizzle_x(q[b].T.astype(NPBF16)) for b in range(B)]
    kT = [_swizzle_x(k[b].T.astype(NPBF16)) for b in range(B)]
    vT = [_swizzle_x(v[b].T.astype(NPBF16)) for b in range(B)]
    woT = _swizzle_x(w_o.astype(NPBF16).T)
    in_maps = []
    for c in range(N_CORES):
        d0 = c * DPC
        m = {}
        for b in range(B):
            m[f"qT{b}"] = qT[b]
            m[f"kT{b}"] = kT[b]
            m[f"vT{b}"] = vT[b]
        m["wqT"] = _tile_weight(
            np.ascontiguousarray(w_q[d0:d0 + DPC, :].T).astype(NPBF16))
        m["wkT"] = _tile_weight(
            np.ascontiguousarray(w_k[d0:d0 + DPC, :].T).astype(NPBF16))
        m["wvT"] = _tile_weight(
            np.ascontiguousarray(w_v[d0:d0 + DPC, :].T).astype(NPBF16))
        m["woT"] = woT
        in_maps.append(m)
    return in_maps


def kernel(q, k, v, mask, w_q, w_k, w_v, w_o, _trace=False):
    q = np.asarray(q, np.float32)
    k = np.asarray(k, np.float32)
    v = np.asarray(v, np.float32)
    mask = np.asarray(mask)
    w_q = np.asarray(w_q, np.float32)
    w_k = np.asarray(w_k, np.float32)
    w_v = np.asarray(w_v, np.float32)
    w_o = np.asarray(w_o, np.float32)
    S = q.shape[1]

    if not np.all(mask != 0):
        # General-mask fallback (never hit for the eval problem: mask is all
        # ones).  Computed on host for correctness.
        return _numpy_reference(q, k, v, mask, w_q, w_k, w_v, w_o)

    if S not in _NC_CACHE:
        _NC_CACHE[S] = build_nc(S)
    nc = _NC_CACHE[S]

    in_maps = make_in_maps(q, k, v, w_q, w_k, w_v, w_o)
    res = run_bass_kernel_spmd(nc, in_maps, core_ids=list(range(N_CORES)),
                               trace=_trace)

    SCB = S // N_CORES
    out = np.empty((B, S, HIDDEN), np.float32)
    for c in range(N_CORES):
        for b in range(B):
            out[b, c * SCB:(c + 1) * SCB, :] = \
                res.results[c]["out"][b * SCB:(b + 1) * SCB, :].astype(
                    np.float32)
    if _trace:
        return out, res
    return out


def _numpy_reference(q, k, v, mask, w_q, w_k, w_v, w_o):
    Bn, S, H = q.shape
    dk = H // HEADS

    def split_heads(x, w):
        y = x @ w.T
        return y.reshape(Bn, S, HEADS, dk).transpose(0, 2, 1, 3)

    qh = split_heads(q, w_q)
    kh = split_heads(k, w_k)
    vh = split_heads(v, w_v)
    s = np.einsum("bhqd,bhkd->bhqk", qh, kh) / np.sqrt(np.float32(dk))
    s = np.where(mask[:, None, :, :] == 0, np.float32(-1e9), s)
    s = s - s.max(-1, keepdims=True)
    e = np.exp(s)
    a = e / e.sum(-1, keepdims=True)
    o = np.einsum("bhqk,bhkd->bhqd", a, vh)
    o = o.transpose(0, 2, 1, 3).reshape(Bn, S, H)
    return (o @ w_o.T).astype(np.float32)
